# revision 1
# baseline (speedup 1.0000x reference)
"""CenterLoss forward on 8 Trainium2 NeuronCores (Bass/Tile).

loss = mean_b ||features[b] - centers[labels[b]]||^2  (LAMBDA_C = 1.0)

Strategy (data-parallel, per the sharding hint):
  - Shard features/labels along batch across 8 cores (8192 rows each);
    replicate centers in every core's HBM.
  - Per core: stream the features shard ([128 partitions x 64 rows] layout,
    64KB contiguous per partition), gather the 8192 label-indexed center rows
    with SWDGE indirect DMA (one [P,1]-offset instruction per slot column —
    the HW consumes one index per partition per instruction), then per-chunk
    DVE subtract + DVE square, ones-matmul accumulation into PSUM for the
    reduction, and DMA out a [1,1] partial sum.
  - Host sums the 8 partials and divides by the full batch (the scalar
    "all-reduce" of the mean).
"""

import numpy as np

import concourse.bacc as bacc
import concourse.bass as bass
import concourse.mybir as mybir
import concourse.tile as tile
from concourse.bass_utils import run_bass_kernel_spmd

NCORES = 8
BATCH = 65536
FEAT_DIM = 256
NUM_CLASSES = 100000
LAMBDA_C = 1.0

SHARD = BATCH // NCORES  # 8192 rows per core
P = 128  # SBUF partitions
G = SHARD // P  # 64 examples per partition
FREE = G * FEAT_DIM  # 16384 f32 per partition (64KB)
NCH = 8  # pipeline chunks
GC = G // NCH  # 8 gathered rows per partition per chunk
CFREE = FREE // NCH  # 2048 f32 per partition per chunk

_f32 = mybir.dt.float32


def _build():
    nc = bacc.Bacc(
        "TRN2",
        target_bir_lowering=False,
        debug=False,
        num_devices=NCORES,
        enable_asserts=False,
        # 2x the default descriptor-ring carveout so SWDGE desc-gen for the
        # 64 serial indirect gathers never blocks on ring space.
        dynamic_dma_scratch_size=32768,
    )
    feat_d = nc.dram_tensor("features", [SHARD, FEAT_DIM], _f32, kind="ExternalInput")
    lab_d = nc.dram_tensor("labels", [SHARD], mybir.dt.int32, kind="ExternalInput")
    cent_d = nc.dram_tensor(
        "centers", [NUM_CLASSES, FEAT_DIM], _f32, kind="ExternalInput"
    )
    out_d = nc.dram_tensor("partial", [1, 1], _f32, kind="ExternalOutput")

    feat_src = feat_d.ap().rearrange("(p g) d -> p (g d)", p=P)
    lab_src = lab_d.ap().rearrange("(p g) -> p g", p=P)

    with tile.TileContext(nc) as tc:
        with (
            tc.tile_pool(name="big", bufs=1) as big,
            tc.tile_pool(name="sc", bufs=2) as sc,
            tc.tile_pool(name="ps", bufs=1, space="PSUM") as ps,
        ):
            # Labels land in per-chunk pieces, issued before any feature DMA,
            # so the first gather's dependency clears as early as possible.
            CHUNK_COLS = [10, 10, 10, 10, 10, 10, 2, 2]
            assert sum(CHUNK_COLS) == G
            offs = [sum(CHUNK_COLS[:j]) for j in range(len(CHUNK_COLS))]
            lab = big.tile([P, G], mybir.dt.int32)
            for c0, ncols in zip(offs, CHUNK_COLS):
                nc.sync.dma_start(
                    out=lab[:, c0 : c0 + ncols], in_=lab_src[:, c0 : c0 + ncols]
                )

            ones = big.tile([P, 1], _f32)
            nc.vector.memset(ones[:], 1.0)

            feat = big.tile([P, FREE], _f32)
            cent = big.tile([P, FREE], _f32)

            # Uneven chunks: the last two are tiny so the serial compute tail
            # after the final gather is short.
            for c0, ncols in zip(offs, CHUNK_COLS):
                sl = slice(c0 * FEAT_DIM, (c0 + ncols) * FEAT_DIM)
                nc.sync.dma_start(out=feat[:, sl], in_=feat_src[:, sl])
                # One indirect DMA per slot-column: the HW SWDGE consumes ONE
                # index per partition per instruction and gathers the whole
                # out-free-size contiguously from that row, so offsets must be
                # [P, 1] (multi-index offset tiles only work in the simulator).
                for g in range(c0, c0 + ncols):
                    nc.gpsimd.indirect_dma_start(
                        out=cent[:, g * FEAT_DIM : (g + 1) * FEAT_DIM],
                        out_offset=None,
                        in_=cent_d.ap(),
                        in_offset=bass.IndirectOffsetOnAxis(
                            ap=lab[:, g : g + 1], axis=0
                        ),
                    )

            # PSUM accumulator row: res_ps[0, n] accumulates
            # sum_p sq[p, k*512 + n] across every chunk via ones-matmuls
            # (plain fp32 PSUM adds; verified ~1e-7 relative on HW).
            MMN = 512
            total_mm = G * FEAT_DIM // MMN
            res_ps = ps.tile([1, MMN], _f32)
            mm = 0
            for c0, ncols in zip(offs, CHUNK_COLS):
                cf = ncols * FEAT_DIM
                sl = slice(c0 * FEAT_DIM, c0 * FEAT_DIM + cf)
                diff = sc.tile([P, cf], _f32, tag="diff")
                nc.vector.tensor_tensor(
                    out=diff[:],
                    in0=feat[:, sl],
                    in1=cent[:, sl],
                    op=mybir.AluOpType.subtract,
                )
                # Square on DVE (exact fp32 multiply; ACT's Square is a
                # piecewise-polynomial approximation on HW).
                sq = sc.tile([P, cf], _f32, tag="sq")
                nc.vector.tensor_tensor(
                    out=sq[:], in0=diff[:], in1=diff[:], op=mybir.AluOpType.mult
                )
                for k in range(cf // MMN):
                    nc.tensor.matmul(
                        out=res_ps[:],
                        lhsT=ones[:],
                        rhs=sq[:, k * MMN : (k + 1) * MMN],
                        start=(mm == 0),
                        stop=(mm == total_mm - 1),
                    )
                    mm += 1

            # Reduce straight out of PSUM on DVE (skips a copy in the tail).
            res_sb = big.tile([1, 1], _f32)
            nc.vector.reduce_sum(
                out=res_sb[:], in_=res_ps[:], axis=mybir.AxisListType.X
            )
            nc.sync.dma_start(out=out_d.ap(), in_=res_sb[:])

    nc.compile()
    return nc


_nc_cache = None


def _get_nc():
    global _nc_cache
    if _nc_cache is None:
        _nc_cache = _build()
    return _nc_cache


def _make_in_maps(features, labels, centers):
    features = np.ascontiguousarray(np.asarray(features, dtype=np.float32))
    labels = np.ascontiguousarray(np.asarray(labels).astype(np.int32))
    centers = np.ascontiguousarray(np.asarray(centers, dtype=np.float32))
    assert features.shape == (BATCH, FEAT_DIM)
    assert labels.shape == (BATCH,)
    assert centers.shape == (NUM_CLASSES, FEAT_DIM)
    return [
        {
            "features": features[k * SHARD : (k + 1) * SHARD],
            "labels": labels[k * SHARD : (k + 1) * SHARD],
            "centers": centers,
        }
        for k in range(NCORES)
    ]


def _reduce_results(results):
    total = sum(float(r["partial"][0, 0]) for r in results)
    return np.float32(LAMBDA_C * total / BATCH)


def kernel(features: np.ndarray, labels: np.ndarray, centers: np.ndarray):
    in_maps = _make_in_maps(features, labels, centers)
    res = run_bass_kernel_spmd(_get_nc(), in_maps, core_ids=list(range(NCORES)))
    return _reduce_results(res.results)



# revision 7
# speedup vs baseline: 1.2309x; 1.2309x over previous
"""CenterLoss forward on 8 Trainium2 NeuronCores (Bass/Tile).

loss = mean_b ||features[b] - centers[labels[b]]||^2  (LAMBDA_C = 1.0)

Strategy — CLASS-RANGE sharding (the loss is a permutation-invariant sum
over examples, so any example->core routing is a valid sharding):
  - Core k owns classes [12500k, 12500(k+1)). The host routes each example
    to the core that owns its label and ships that core only its 12500-row
    slice of the centers table. Local class indices are < 12500, so they
    fit int16 — which unlocks the gpsimd `dma_gather` custom instruction:
    one instruction gathers THOUSANDS of center rows (994ns fixed +
    0.34ns/descriptor) instead of the 128-rows-per-instruction indirect
    DMA (994ns fixed each) that made the baseline's gpsimd descriptor
    generation the bottleneck (112us busy on an 8MB gather).
  - Shards are padded to a common row count; pad rows use local class 0
    with the pad feature row set to that exact center row, so they
    contribute exactly 0 to the sum.
  - Data is shipped as bf16 (tolerance gate is 2e-2; measured rel err
    ~4e-5), halving HBM traffic: ~4.3MB features + ~4.3MB gathered
    centers per core.
  - Per chunk: DVE subtract, DVE square, then ones-matmul accumulation
    of every 512-wide slice into one PSUM row (f32 accumulate). Final
    reduction: DVE reduce_sum straight out of PSUM. (tensor_tensor_reduce
    would fuse square+reduce in one DVE pass but its ISA opcode crashes
    this runtime — verified with a minimal HW probe.)
  - Host sums the 8 partial scalars and divides by the batch size.
"""

import ml_dtypes
import numpy as np

import concourse.bacc as bacc
import concourse.mybir as mybir
import concourse.tile as tile
from concourse.bass_utils import run_bass_kernel_spmd

NCORES = 8
BATCH = 65536
FEAT_DIM = 256
NUM_CLASSES = 100000
CSHARD = NUM_CLASSES // NCORES  # 12500 classes per core
LAMBDA_C = 1.0
P = 128

USE_BF16 = True
_dt = mybir.dt.bfloat16 if USE_BF16 else mybir.dt.float32
_np_dt = ml_dtypes.bfloat16 if USE_BF16 else np.float32
_f32 = mybir.dt.float32


def _chunks(nrb):
    """Split nrb 128-row blocks into 8-block (1024-row) chunks. The HW
    dma_gather ucode rejects more than 1024 indices per instruction
    (empirical: 1024 OK, 1040 crashes, independent of the SWDGE scratch
    size)."""
    out = []
    b0 = 0
    while b0 < nrb:
        cb = min(8, nrb - b0)
        out.append((b0, cb))
        b0 += cb
    return out


def _build(nrb):
    nc = bacc.Bacc(
        "TRN2",
        target_bir_lowering=False,
        debug=False,
        num_devices=NCORES,
        enable_asserts=False,
        # 3x the default SWDGE descriptor-ring carveout so several 1024-row
        # gathers can be in flight while the next one's descriptors generate.
        dynamic_dma_scratch_size=49152,
    )
    feat_d = nc.dram_tensor("features", [P, nrb, FEAT_DIM], _dt, kind="ExternalInput")
    lab_d = nc.dram_tensor("labels", [P, nrb * 8], mybir.dt.int16, kind="ExternalInput")
    cent_d = nc.dram_tensor("centers", [CSHARD, FEAT_DIM], _dt, kind="ExternalInput")
    out_d = nc.dram_tensor("partial", [1, 1], _f32, kind="ExternalOutput")

    chunks = _chunks(nrb)
    nch = len(chunks)

    with tile.TileContext(nc) as tc:
        with (
            tc.tile_pool(name="big", bufs=1) as big,
            tc.tile_pool(name="io", bufs=3) as io,
            tc.tile_pool(name="sc", bufs=2) as sc,
            tc.tile_pool(name="ps", bufs=1, space="PSUM") as ps,
        ):
            # Gather indices for the whole shard, wrapped [16, nr/16] and
            # replicated to 128 partitions (dma_gather's expected layout).
            lab = big.tile([P, nrb * 8], mybir.dt.int16)
            nc.sync.dma_start(out=lab[:], in_=lab_d.ap())

            ones = big.tile([P, 1], _dt)
            nc.vector.memset(ones[:], 1.0)

            MMB = 2  # 512-elem (2-block) matmul slices; PSUM bank = 512 f32
            total_mm = sum(-(-cb // MMB) for _, cb in chunks)
            res_ps = ps.tile([1, MMB * FEAT_DIM], _f32)
            mm = 0

            for c, (b0, cb) in enumerate(chunks):
                feat_t = io.tile([P, cb, FEAT_DIM], _dt, tag=f"feat{cb}")
                cent_t = io.tile([P, cb, FEAT_DIM], _dt, tag=f"cent{cb}")
                nc.sync.dma_start(out=feat_t[:], in_=feat_d.ap()[:, b0 : b0 + cb, :])
                # One SWDGE instruction gathers all cb*128 center rows; row i
                # lands at [i%128, i//128, :], matching the host's feature
                # wrap layout.
                nc.gpsimd.dma_gather(
                    cent_t[:],
                    cent_d.ap(),
                    lab[:, b0 * 8 : (b0 + cb) * 8],
                    cb * P,
                    cb * P,
                    FEAT_DIM,
                )
                diff_t = sc.tile([P, cb, FEAT_DIM], _dt, tag=f"diff{cb}")
                nc.vector.tensor_tensor(
                    out=diff_t[:],
                    in0=feat_t[:],
                    in1=cent_t[:],
                    op=mybir.AluOpType.subtract,
                )
                # Square on DVE (exact product of bf16 values), then reduce
                # via ones-matmuls: PSUM accumulates in f32.
                sq_t = sc.tile([P, cb, FEAT_DIM], _dt, tag=f"sq{cb}")
                nc.vector.tensor_tensor(
                    out=sq_t[:],
                    in0=diff_t[:],
                    in1=diff_t[:],
                    op=mybir.AluOpType.mult,
                )
                for j0 in range(0, cb, MMB):
                    nb = min(MMB, cb - j0)
                    nc.tensor.matmul(
                        out=res_ps[:, : nb * FEAT_DIM],
                        lhsT=ones[:],
                        rhs=sq_t[:, j0 : j0 + nb, :],
                        start=(mm == 0),
                        stop=(mm == total_mm - 1),
                    )
                    mm += 1

            # [1, 512] PSUM -> [1, 1] -> HBM
            res_sb = big.tile([1, 1], _f32)
            nc.vector.reduce_sum(out=res_sb[:], in_=res_ps[:], axis=mybir.AxisListType.X)
            nc.sync.dma_start(out=out_d.ap(), in_=res_sb[:])

    nc.compile()
    return nc


_nc_cache = {}


def _get_nc(nrb):
    if nrb not in _nc_cache:
        _nc_cache[nrb] = _build(nrb)
    return _nc_cache[nrb]


def _make_in_maps(features, labels, centers):
    features = np.ascontiguousarray(np.asarray(features, dtype=np.float32))
    labels = np.ascontiguousarray(np.asarray(labels)).astype(np.int64)
    centers = np.ascontiguousarray(np.asarray(centers, dtype=np.float32))
    assert features.shape == (BATCH, FEAT_DIM)
    assert labels.shape == (BATCH,)
    assert centers.shape == (NUM_CLASSES, FEAT_DIM)

    bucket = labels // CSHARD
    order = np.argsort(bucket, kind="stable")
    counts = np.bincount(bucket, minlength=NCORES)
    # Blocks of 128 rows; at least 2 so every chunk list is non-degenerate.
    nrb = max(2, -(-int(counts.max()) // P))
    nr = nrb * P

    cent_np = centers.astype(_np_dt)
    in_maps = []
    pos = 0
    for k in range(NCORES):
        n = int(counts[k])
        idx = order[pos : pos + n]
        pos += n
        cshard = cent_np[k * CSHARD : (k + 1) * CSHARD]
        feat_k = np.empty((nr, FEAT_DIM), dtype=_np_dt)
        feat_k[:n] = features[idx].astype(_np_dt)
        # Pad rows: local class 0 with its exact center row -> diff == 0.
        feat_k[n:] = cshard[0]
        loc = np.zeros((nr,), dtype=np.int16)
        loc[:n] = (labels[idx] - k * CSHARD).astype(np.int16)
        # dma_gather index layout: index i at [i%16, i//16], replicated to
        # all 128 partitions.
        lab16 = np.ascontiguousarray(
            np.tile(loc.reshape(nr // 16, 16).T, (P // 16, 1))
        )
        # Row i -> partition i%128, block i//128 (matches gather output).
        featw = np.ascontiguousarray(
            feat_k.reshape(nrb, P, FEAT_DIM).transpose(1, 0, 2)
        )
        in_maps.append({"features": featw, "labels": lab16, "centers": cshard})
    return in_maps, nrb


def _reduce_results(results):
    total = sum(float(r["partial"][0, 0]) for r in results)
    return np.float32(LAMBDA_C * total / BATCH)


def kernel(features: np.ndarray, labels: np.ndarray, centers: np.ndarray):
    in_maps, nrb = _make_in_maps(features, labels, centers)
    res = run_bass_kernel_spmd(_get_nc(nrb), in_maps, core_ids=list(range(NCORES)))
    return _reduce_results(res.results)


# revision 10
# speedup vs baseline: 1.7033x; 1.3838x over previous
"""CenterLoss forward on 8 Trainium2 NeuronCores (Bass/Tile).

loss = mean_b ||features[b] - centers[labels[b]]||^2  (LAMBDA_C = 1.0)

Strategy — CLASS-RANGE sharding (the loss is a permutation-invariant sum
over examples, so any example->core routing is a valid sharding):
  - Core k owns classes [12500k, 12500(k+1)). The host routes each example
    to the core that owns its label and ships that core only its 12500-row
    slice of the centers table. Local class indices are < 12500, so they
    fit int16 — which unlocks the gpsimd `dma_gather` custom instruction:
    one instruction gathers THOUSANDS of center rows (994ns fixed +
    0.34ns/descriptor) instead of the 128-rows-per-instruction indirect
    DMA (994ns fixed each) that made the baseline's gpsimd descriptor
    generation the bottleneck (112us busy on an 8MB gather).
  - Shards are padded to a common row count; pad rows use local class 0
    with the pad feature row set to that exact center row, so they
    contribute exactly 0 to the sum.
  - Data is shipped as bf16 (tolerance gate is 2e-2; measured rel err
    ~4e-5), halving HBM traffic: ~4.3MB features + ~4.3MB gathered
    centers per core.
  - Per chunk: DVE subtract, DVE square, then ones-matmul accumulation
    of every 512-wide slice into one PSUM row (f32 accumulate). Final
    reduction: DVE reduce_sum straight out of PSUM. (tensor_tensor_reduce
    would fuse square+reduce in one DVE pass but its ISA opcode crashes
    this runtime — verified with a minimal HW probe.)
  - Host sums the 8 partial scalars and divides by the batch size.
"""

import ml_dtypes
import numpy as np

import concourse.bacc as bacc
import concourse.mybir as mybir
import concourse.tile as tile
from concourse.bass_utils import run_bass_kernel_spmd

NCORES = 8
BATCH = 65536
FEAT_DIM = 256
NUM_CLASSES = 100000
CSHARD = NUM_CLASSES // NCORES  # 12500 classes per core
LAMBDA_C = 1.0
P = 128

USE_BF16 = True
_dt = mybir.dt.bfloat16 if USE_BF16 else mybir.dt.float32
_np_dt = ml_dtypes.bfloat16 if USE_BF16 else np.float32
_f32 = mybir.dt.float32


def _chunks(nrb):
    """Split nrb 128-row blocks into 8-block (1024-row) chunks. The HW
    dma_gather ucode rejects more than 1024 indices per instruction
    (empirical: 1024 OK, 1040 crashes, independent of the SWDGE scratch
    size)."""
    out = []
    b0 = 0
    while b0 < nrb:
        cb = min(8, nrb - b0)
        out.append((b0, cb))
        b0 += cb
    return out


def _build(nrb):
    nc = bacc.Bacc(
        "TRN2",
        target_bir_lowering=False,
        debug=False,
        num_devices=NCORES,
        enable_asserts=False,
        # 3x the default SWDGE descriptor-ring carveout so several 1024-row
        # gathers can be in flight while the next one's descriptors generate.
        dynamic_dma_scratch_size=49152,
        # One ring per queue: alternating queues lets gather i+1's descriptor
        # generation overlap gather i's DMA drain (each queue has its own
        # 1024-descriptor staging, which is also why num_idxs caps at 1024).
        num_swdge_queues=4,
    )
    feat_d = nc.dram_tensor("features", [P, nrb, FEAT_DIM], _dt, kind="ExternalInput")
    lab_d = nc.dram_tensor("labels", [P, nrb * 8], mybir.dt.int16, kind="ExternalInput")
    cent_d = nc.dram_tensor("centers", [CSHARD, FEAT_DIM], _dt, kind="ExternalInput")
    out_d = nc.dram_tensor("partial", [1, 1], _f32, kind="ExternalOutput")

    chunks = _chunks(nrb)
    nch = len(chunks)

    with tile.TileContext(nc) as tc:
        with (
            tc.tile_pool(name="big", bufs=1) as big,
            tc.tile_pool(name="io", bufs=3) as io,
            tc.tile_pool(name="sc", bufs=2) as sc,
            tc.tile_pool(name="ps", bufs=1, space="PSUM") as ps,
        ):
            # Gather indices for the whole shard, wrapped [16, nr/16] and
            # replicated to 128 partitions (dma_gather's expected layout).
            lab = big.tile([P, nrb * 8], mybir.dt.int16)
            nc.sync.dma_start(out=lab[:], in_=lab_d.ap())

            # Warmup gather: absorbs the one-time Q7 custom-ucode library
            # load (~12us) while the label DMA is in flight.
            warm_idx = big.tile([P, 1], mybir.dt.int16)
            nc.vector.memset(warm_idx[:], 0)
            warm_out = big.tile([P, 1, FEAT_DIM], _dt)
            nc.gpsimd.dma_gather(
                warm_out[:], cent_d.ap(), warm_idx[:], 16, 16, FEAT_DIM
            )

            ones = big.tile([P, 1], _dt)
            nc.vector.memset(ones[:], 1.0)

            MMB = 2  # 512-elem (2-block) matmul slices; PSUM bank = 512 f32
            total_mm = sum(-(-cb // MMB) for _, cb in chunks)
            res_ps = ps.tile([1, MMB * FEAT_DIM], _f32)
            mm = 0

            for c, (b0, cb) in enumerate(chunks):
                feat_t = io.tile([P, cb, FEAT_DIM], _dt, tag=f"feat{cb}")
                cent_t = io.tile([P, cb, FEAT_DIM], _dt, tag=f"cent{cb}")
                nc.sync.dma_start(out=feat_t[:], in_=feat_d.ap()[:, b0 : b0 + cb, :])
                # One SWDGE instruction gathers all cb*128 center rows; row i
                # lands at [i%128, i//128, :], matching the host's feature
                # wrap layout.
                nc.gpsimd.dma_gather(
                    cent_t[:],
                    cent_d.ap(),
                    lab[:, b0 * 8 : (b0 + cb) * 8],
                    cb * P,
                    cb * P,
                    FEAT_DIM,
                    queue_num=c % 4,
                )
                diff_t = sc.tile([P, cb, FEAT_DIM], _dt, tag=f"diff{cb}")
                nc.vector.tensor_tensor(
                    out=diff_t[:],
                    in0=feat_t[:],
                    in1=cent_t[:],
                    op=mybir.AluOpType.subtract,
                )
                # Square on DVE (exact product of bf16 values), then reduce
                # via ones-matmuls: PSUM accumulates in f32.
                sq_t = sc.tile([P, cb, FEAT_DIM], _dt, tag=f"sq{cb}")
                nc.vector.tensor_tensor(
                    out=sq_t[:],
                    in0=diff_t[:],
                    in1=diff_t[:],
                    op=mybir.AluOpType.mult,
                )
                for j0 in range(0, cb, MMB):
                    nb = min(MMB, cb - j0)
                    nc.tensor.matmul(
                        out=res_ps[:, : nb * FEAT_DIM],
                        lhsT=ones[:],
                        rhs=sq_t[:, j0 : j0 + nb, :],
                        start=(mm == 0),
                        stop=(mm == total_mm - 1),
                    )
                    mm += 1

            # [1, 512] PSUM -> [1, 1] -> HBM
            res_sb = big.tile([1, 1], _f32)
            nc.vector.reduce_sum(out=res_sb[:], in_=res_ps[:], axis=mybir.AxisListType.X)
            nc.sync.dma_start(out=out_d.ap(), in_=res_sb[:])

    nc.compile()
    return nc


_nc_cache = {}


def _get_nc(nrb):
    if nrb not in _nc_cache:
        _nc_cache[nrb] = _build(nrb)
    return _nc_cache[nrb]


def _make_in_maps(features, labels, centers):
    features = np.ascontiguousarray(np.asarray(features, dtype=np.float32))
    labels = np.ascontiguousarray(np.asarray(labels)).astype(np.int64)
    centers = np.ascontiguousarray(np.asarray(centers, dtype=np.float32))
    assert features.shape == (BATCH, FEAT_DIM)
    assert labels.shape == (BATCH,)
    assert centers.shape == (NUM_CLASSES, FEAT_DIM)

    bucket = labels // CSHARD
    order = np.argsort(bucket, kind="stable")
    counts = np.bincount(bucket, minlength=NCORES)
    # Blocks of 128 rows; at least 2 so every chunk list is non-degenerate.
    nrb = max(2, -(-int(counts.max()) // P))
    nr = nrb * P

    cent_np = centers.astype(_np_dt)
    in_maps = []
    pos = 0
    for k in range(NCORES):
        n = int(counts[k])
        idx = order[pos : pos + n]
        pos += n
        cshard = cent_np[k * CSHARD : (k + 1) * CSHARD]
        feat_k = np.empty((nr, FEAT_DIM), dtype=_np_dt)
        feat_k[:n] = features[idx].astype(_np_dt)
        # Pad rows: local class 0 with its exact center row -> diff == 0.
        feat_k[n:] = cshard[0]
        loc = np.zeros((nr,), dtype=np.int16)
        loc[:n] = (labels[idx] - k * CSHARD).astype(np.int16)
        # dma_gather index layout: index i at [i%16, i//16], replicated to
        # all 128 partitions.
        lab16 = np.ascontiguousarray(
            np.tile(loc.reshape(nr // 16, 16).T, (P // 16, 1))
        )
        # Row i -> partition i%128, block i//128 (matches gather output).
        featw = np.ascontiguousarray(
            feat_k.reshape(nrb, P, FEAT_DIM).transpose(1, 0, 2)
        )
        in_maps.append({"features": featw, "labels": lab16, "centers": cshard})
    return in_maps, nrb


def _reduce_results(results):
    total = sum(float(r["partial"][0, 0]) for r in results)
    return np.float32(LAMBDA_C * total / BATCH)


def kernel(features: np.ndarray, labels: np.ndarray, centers: np.ndarray):
    in_maps, nrb = _make_in_maps(features, labels, centers)
    res = run_bass_kernel_spmd(_get_nc(nrb), in_maps, core_ids=list(range(NCORES)))
    return _reduce_results(res.results)
